# revision 7
# baseline (speedup 1.0000x reference)
"""BiMPM (bilateral multi-perspective matching) kernel for Trainium2.

Contract: kernel(**inputs) takes the FULL unsharded inputs (as produced by
setup_inputs) and returns the full [B, 2L, 102] output. Internally shards
data-parallel over batch B=8 across 8 NeuronCores; the tiny perspective
weights are folded host-side into per-core aux tensors.

Self-contained: hardcodes B=8, L=128, H=768, P=16.
"""
import sys

sys.path.insert(0, "/opt/trn_rl_repo")

import numpy as np
from contextlib import ExitStack

from concourse import bacc, mybir, masks
import concourse.tile as tile
from concourse.bass_utils import run_bass_kernel_spmd
from concourse.bass import MemorySpace

B, L, H, PP, NCH, NF = 8, 128, 768, 16, 6, 102
EPS = 1e-8
F32 = mybir.dt.float32
BF16 = mybir.dt.bfloat16
AX = mybir.AxisListType
OP = mybir.AluOpType
AF = mybir.ActivationFunctionType

# w2t column blocks: [ones|ff16 | ones|fb16 | ones|att16 | ones|matt16 | mp16]
BLK_FF = slice(0, 17)
BLK_FB = slice(17, 34)
BLK_ATT = slice(34, 51)
BLK_MATT = slice(51, 68)
MP0 = 68


def _trace_kernel(tc, dins, dout):
    nc = tc.nc
    with ExitStack() as ctx:
        sb = ctx.enter_context(tc.tile_pool(name="sb", bufs=1))
        sc = ctx.enter_context(tc.tile_pool(name="sc", bufs=2))
        ps_t = ctx.enter_context(
            tc.tile_pool(name="ps_t", bufs=4, space=MemorySpace.PSUM))
        ps_w = ctx.enter_context(
            tc.tile_pool(name="ps_w", bufs=2, space=MemorySpace.PSUM))

        # ---- load inputs ----
        def load(name, shape, rearr=None, **kw):
            t = sb.tile(shape, F32, tag=name)
            src = dins[name][:]
            if rearr is not None:
                src = src.rearrange(rearr, **kw)
            nc.sync.dma_start(t[:], src)
            return t

        c1 = load("c1", [L, H])
        c2 = load("c2", [L, H])
        w2t = load("w2t", [L, NCH, 84], "(c p) n -> p c n", p=L)
        rhs1 = load("rhs1", [L, NCH, 34], "(c p) n -> p c n", p=L)
        rhs2 = load("rhs2", [L, NCH, 34], "(c p) n -> p c n", p=L)
        mneg1b = load("mneg1b", [L, L])
        mneg2b = load("mneg2b", [L, L])
        mone1b = load("mone1b", [L, L])
        mone2b = load("mone2b", [L, L])
        mnegc1 = load("mnegc1", [L, 1])
        mnegc2 = load("mnegc2", [L, 1])
        invl1 = load("invl1", [L, 1])
        invl2 = load("invl2", [L, 1])

        ident = sb.tile([L, L], F32, tag="ident")
        masks.make_identity(nc, ident[:])
        identb = sb.tile([L, L], BF16, tag="identb")
        masks.make_identity(nc, identb[:])
        ones_row = sb.tile([1, L], F32, tag="ones_row")
        nc.vector.memset(ones_row[:], 1.0)
        ones_col = sb.tile([L, 1], F32, tag="ones_col")
        nc.vector.memset(ones_col[:], 1.0)

        out1 = sb.tile([L, NF], F32, tag="out1")
        out2 = sb.tile([L, NF], F32, tag="out2")

        # ---- c1x/c2x: -1e30 rows at invalid positions ----
        c1x = sb.tile([L, H], F32, tag="c1x")
        nc.vector.tensor_scalar(c1x[:], c1[:], mnegc1[:, 0:1], None, OP.add)
        c2x = sb.tile([L, H], F32, tag="c2x")
        nc.vector.tensor_scalar(c2x[:], c2[:], mnegc2[:, 0:1], None, OP.add)

        # ---- transposes + squares ----
        c1T = sb.tile([L, NCH, L], F32, tag="c1T")
        c1sqT = sb.tile([L, NCH, L], F32, tag="c1sqT")
        c2T = sb.tile([L, NCH, L], F32, tag="c2T")
        c2sqT = sb.tile([L, NCH, L], F32, tag="c2sqT")
        for (src, dT, dsqT) in ((c1, c1T, c1sqT), (c2, c2T, c2sqT)):
            for c in range(NCH):
                tp = ps_t.tile([L, L], F32, tag="t")
                nc.tensor.transpose(tp[:], src[:, c * L:(c + 1) * L], ident[:])
                nc.scalar.copy(dT[:, c, :], tp[:])
                nc.scalar.square(dsqT[:, c, :], tp[:])

        # ---- weighted norms -> rw1/rw2 [L,84] ----
        def rw_of(sqT, tag):
            wnp = ps_w.tile([L, 84], F32, tag="w")
            for c in range(NCH):
                nc.tensor.matmul(wnp[:], sqT[:, c, :], w2t[:, c, :],
                                 start=(c == 0), stop=(c == NCH - 1))
            rw = sb.tile([L, 84], F32, tag=tag)
            nc.scalar.sqrt(rw[:], wnp[:])
            nc.vector.tensor_scalar(rw[:], rw[:], EPS, None, OP.max)
            nc.vector.reciprocal(rw[:], rw[:])
            return rw

        rw1 = rw_of(c1sqT, "rw1")
        rw2 = rw_of(c2sqT, "rw2")

        # ---- ff/bf matvec features -> out[:, 2:36] ----
        def ff_feats(cT, rhs, rw, out):
            ffp = ps_w.tile([L, 34], F32, tag="w")
            for c in range(NCH):
                nc.tensor.matmul(ffp[:], cT[:, c, :], rhs[:, c, :],
                                 start=(c == 0), stop=(c == NCH - 1))
            nc.vector.tensor_tensor(out[:, 2:36], ffp[:], rw[:, 0:34], op=OP.mult)

        ff_feats(c1T, rhs1, rw1, out1)
        ff_feats(c2T, rhs2, rw2, out2)

        # ---- cos chain ----
        dotsp = ps_t.tile([L, L], F32, tag="t")
        for c in range(NCH):
            nc.tensor.matmul(dotsp[:], c1T[:, c, :], c2T[:, c, :],
                             start=(c == 0), stop=(c == NCH - 1))
        wS = sc.tile([L, L], F32, tag="wS")
        nc.vector.tensor_scalar(wS[:], dotsp[:], rw1[:, 0:1], None, OP.mult)
        wTp = ps_t.tile([L, L], F32, tag="t")
        nc.tensor.transpose(wTp[:], wS[:], ident[:])
        cosT = sb.tile([L, L], F32, tag="cosT")
        nc.vector.tensor_scalar(cosT[:], wTp[:], rw2[:, 0:1], None, OP.mult)
        cosp = ps_t.tile([L, L], F32, tag="t")
        nc.tensor.transpose(cosp[:], cosT[:], ident[:])
        cos = sb.tile([L, L], F32, tag="cos")
        nc.scalar.copy(cos[:], cosp[:])

        # ---- cmax / cmean -> out[:, 0:2] ----
        def cmaxmean(cosA, cosB, mnegb, invl, out):
            # cosA [a,b]; max/mean over b (free); cosB = cosA^T for the matvec
            t = sc.tile([L, L], F32, tag="cm")
            nc.vector.tensor_tensor(t[:], cosA[:], mnegb[:], op=OP.add)
            nc.vector.reduce_max(out[:, 0:1], t[:], axis=AX.X)
            mp = ps_t.tile([L, 1], F32, tag="t")
            nc.tensor.matmul(mp[:], cosB[:], ones_col[:], start=True, stop=True)
            nc.vector.tensor_scalar(out[:, 1:2], mp[:], invl[:, 0:1], None, OP.mult)

        cmaxmean(cos, cosT, mneg2b, invl2, out1)
        cmaxmean(cosT, cos, mneg1b, invl1, out2)

        # ---- cosM / cosMT (1.0 in invalid columns, for att-max loops) ----
        cosM = sb.tile([L, L], F32, tag="cosM")
        nc.vector.tensor_tensor(cosM[:], cos[:], mone2b[:], op=OP.add)
        cosMT = sb.tile([L, L], F32, tag="cosMT")
        nc.vector.tensor_tensor(cosMT[:], cosT[:], mone1b[:], op=OP.add)

        # ---- attentive mean (softmax over H of cos @ ctx) ----
        def att_mean(lhsT, rhs, tag):
            sp = ps_w.tile([L, H], F32, tag="w")
            nc.tensor.matmul(sp[:, 0:512], lhsT[:], rhs[:, 0:512],
                             start=True, stop=True)
            nc.tensor.matmul(sp[:, 512:H], lhsT[:], rhs[:, 512:H],
                             start=True, stop=True)
            mx = sc.tile([L, 1], F32, tag="mx")
            nc.vector.reduce_max(mx[:], sp[:], axis=AX.X)
            ngm = sc.tile([L, 1], F32, tag="ngm")
            nc.scalar.mul(ngm[:], mx[:], -1.0)
            am = sb.tile([L, H], F32, tag=tag)
            se = sc.tile([L, 1], F32, tag="se")
            nc.scalar.activation(am[:], sp[:], AF.Exp, bias=ngm[:, 0:1],
                                 scale=1.0, accum_out=se[:, 0:1])
            rse = sc.tile([L, 1], F32, tag="rse")
            nc.vector.reciprocal(rse[:], se[:])
            nc.vector.tensor_scalar(am[:], am[:], rse[:, 0:1], None, OP.mult)
            return am

        am2 = att_mean(cosT, c2, "am2")   # [i,H]
        am1 = att_mean(cos, c1, "am1")    # [j,H]

        # ---- attentive max (the big loops) ----
        def att_max(cx, cosMcols, tag):
            cxb = sb.tile([L, H], BF16, tag=tag + "_b")
            nc.vector.tensor_copy(cxb[:], cx[:])
            acc = sb.tile([L, H], F32, tag=tag)
            nc.vector.memset(acc[:], -1e30)
            for j in range(L):
                bc = ps_w.tile([L, H], F32, tag="w")
                sel = identb[:, j:j + 1].to_broadcast([L, L])
                nc.tensor.matmul(bc[:, 0:512], sel, cxb[:, 0:512],
                                 start=True, stop=True)
                nc.tensor.matmul(bc[:, 512:H], sel, cxb[:, 512:H],
                                 start=True, stop=True)
                nc.vector.scalar_tensor_tensor(
                    acc[:], bc[:], cosMcols[:, j:j + 1], acc[:], OP.mult, OP.max)
            return acc

        amx2 = att_max(c2x, cosM, "amx2")    # [i,H]
        amx1 = att_max(c1x, cosMT, "amx1")   # [j,H]

        # ---- mm (pairwise multi-perspective) block ----
        for p in range(PP):
            wcol = w2t[:, :, MP0 + p:MP0 + p + 1]  # per-chunk per-partition scalar
            wc1T = sc.tile([L, NCH, L], F32, tag="wc1T")
            for c in range(NCH):
                if c % 2 == 0:
                    nc.vector.tensor_scalar(wc1T[:, c, :], c1T[:, c, :],
                                            w2t[:, c, MP0 + p:MP0 + p + 1],
                                            None, OP.mult)
                else:
                    nc.scalar.mul(wc1T[:, c, :], c1T[:, c, :],
                                  w2t[:, c, MP0 + p:MP0 + p + 1])
            nump = ps_t.tile([L, L], F32, tag="t")
            for c in range(NCH):
                nc.tensor.matmul(nump[:], wc1T[:, c, :], c2T[:, c, :],
                                 start=(c == 0), stop=(c == NCH - 1))
            numS = sc.tile([L, L], F32, tag="numS")
            nc.scalar.copy(numS[:], nump[:])
            numTp = ps_t.tile([L, L], F32, tag="t")
            nc.tensor.transpose(numTp[:], numS[:], ident[:])
            # side 1: scale cols by rnpc2 (via transposed), max/mean over j
            uT = sc.tile([L, L], F32, tag="uT")
            nc.vector.tensor_scalar(uT[:], numTp[:], rw2[:, MP0 + p:MP0 + p + 1],
                                    None, OP.mult)
            up = ps_t.tile([L, L], F32, tag="t")
            nc.tensor.transpose(up[:], uT[:], ident[:])
            tm1 = sc.tile([L, L], F32, tag="tm1")
            nc.vector.tensor_tensor(tm1[:], up[:], mneg2b[:], op=OP.add)
            m1r = sc.tile([L, 1], F32, tag="m1r")
            nc.vector.reduce_max(m1r[:], tm1[:], axis=AX.X)
            nc.vector.tensor_scalar(out1[:, 36 + p:37 + p], m1r[:],
                                    rw1[:, MP0 + p:MP0 + p + 1], None, OP.mult)
            mn1 = ps_t.tile([L, 1], F32, tag="t")
            nc.tensor.matmul(mn1[:], uT[:], ones_col[:], start=True, stop=True)
            nc.vector.tensor_scalar(out1[:, 52 + p:53 + p], mn1[:],
                                    rw1[:, MP0 + p:MP0 + p + 1], invl2[:, 0:1],
                                    OP.mult, OP.mult)
            # side 2: scale rows by rnpc1, transpose, max/mean over i
            vS = sc.tile([L, L], F32, tag="vS")
            nc.vector.tensor_scalar(vS[:], numS[:], rw1[:, MP0 + p:MP0 + p + 1],
                                    None, OP.mult)
            vTp = ps_t.tile([L, L], F32, tag="t")
            nc.tensor.transpose(vTp[:], vS[:], ident[:])
            tm2 = sc.tile([L, L], F32, tag="tm2")
            nc.vector.tensor_tensor(tm2[:], vTp[:], mneg1b[:], op=OP.add)
            m2r = sc.tile([L, 1], F32, tag="m2r")
            nc.vector.reduce_max(m2r[:], tm2[:], axis=AX.X)
            nc.vector.tensor_scalar(out2[:, 36 + p:37 + p], m2r[:],
                                    rw2[:, MP0 + p:MP0 + p + 1], None, OP.mult)
            mn2 = ps_t.tile([L, 1], F32, tag="t")
            nc.tensor.matmul(mn2[:], vS[:], ones_col[:], start=True, stop=True)
            nc.vector.tensor_scalar(out2[:, 52 + p:53 + p], mn2[:],
                                    rw2[:, MP0 + p:MP0 + p + 1], invl1[:, 0:1],
                                    OP.mult, OP.mult)

        # ---- am/amx rowwise mpm feature blocks ----
        def mpm_block(v, cT, rw_side, blk, out, col0):
            vT = sc.tile([L, NCH, L], F32, tag="vT")
            vsqT = sc.tile([L, NCH, L], F32, tag="vsqT")
            prT = sc.tile([L, NCH, L], F32, tag="prT")
            for c in range(NCH):
                tp = ps_t.tile([L, L], F32, tag="t")
                nc.tensor.transpose(tp[:], v[:, c * L:(c + 1) * L], ident[:])
                nc.scalar.copy(vT[:, c, :], tp[:])
                nc.scalar.square(vsqT[:, c, :], tp[:])
                nc.vector.tensor_tensor(prT[:, c, :], cT[:, c, :], vT[:, c, :],
                                        op=OP.mult)
            nump = ps_w.tile([L, 17], F32, tag="w")
            wnp = ps_w.tile([L, 17], F32, tag="w")
            for c in range(NCH):
                nc.tensor.matmul(nump[:], prT[:, c, :], w2t[:, c, blk],
                                 start=(c == 0), stop=(c == NCH - 1))
            for c in range(NCH):
                nc.tensor.matmul(wnp[:], vsqT[:, c, :], w2t[:, c, blk],
                                 start=(c == 0), stop=(c == NCH - 1))
            rwv = sc.tile([L, 17], F32, tag="rwv")
            nc.scalar.sqrt(rwv[:], wnp[:])
            nc.vector.tensor_scalar(rwv[:], rwv[:], EPS, None, OP.max)
            nc.vector.reciprocal(rwv[:], rwv[:])
            ft = sc.tile([L, 17], F32, tag="ft")
            nc.vector.tensor_tensor(ft[:], nump[:], rw_side[:, blk], op=OP.mult)
            nc.vector.tensor_tensor(out[:, col0:col0 + 17], ft[:], rwv[:],
                                    op=OP.mult)

        mpm_block(am2, c1T, rw1, BLK_ATT, out1, 68)
        mpm_block(am1, c2T, rw2, BLK_ATT, out2, 68)
        mpm_block(amx2, c1T, rw1, BLK_MATT, out1, 85)
        mpm_block(amx1, c2T, rw2, BLK_MATT, out2, 85)

        # ---- store ----
        nc.sync.dma_start(dout[0:L, :], out1[:])
        nc.sync.dma_start(dout[L:2 * L, :], out2[:])


_CACHED = None


def _build():
    global _CACHED
    if _CACHED is not None:
        return _CACHED
    nc = bacc.Bacc("TRN2", target_bir_lowering=False, debug=False,
                   enable_asserts=False)
    dins = {}
    for name, shape in [("c1", [L, H]), ("c2", [L, H]),
                        ("rhs1", [H, 34]), ("rhs2", [H, 34]),
                        ("w2t", [H, 84]),
                        ("mneg1b", [L, L]), ("mneg2b", [L, L]),
                        ("mone1b", [L, L]), ("mone2b", [L, L]),
                        ("mnegc1", [L, 1]), ("mnegc2", [L, 1]),
                        ("invl1", [L, 1]), ("invl2", [L, 1])]:
        dins[name] = nc.dram_tensor(name, shape, F32, kind="ExternalInput")
    dout = nc.dram_tensor("out", [2 * L, NF], F32, kind="ExternalOutput")
    with tile.TileContext(nc) as tc:
        _trace_kernel(tc, dins, dout[:])
    nc.compile()
    _CACHED = nc
    return nc


def _host_prep(c1raw, m1, c2raw, m2, w_ff, w_fb, w_mp, w_att, w_matt):
    c1 = (c1raw * m1[:, None]).astype(np.float32)
    c2 = (c2raw * m2[:, None]).astype(np.float32)
    len1, len2 = float(m1.sum()), float(m2.sum())
    lp1, lp2 = max(int(len1) - 1, 0), max(int(len2) - 1, 0)

    def mpm_rhs(v, w):
        w2 = w * w
        rn = 1.0 / max(np.sqrt((v * v).sum()), EPS)
        wn = np.sqrt((w2 * (v * v)[None, :]).sum(1))
        rwn = 1.0 / np.maximum(wn, EPS)
        return np.concatenate(
            [(v * rn)[:, None], (w2 * v[None, :] * rwn[:, None]).T], 1)

    rhs1 = np.concatenate([mpm_rhs(c2[lp2], w_ff), mpm_rhs(c2[0], w_fb)], 1)
    rhs2 = np.concatenate([mpm_rhs(c1[lp1], w_ff), mpm_rhs(c1[0], w_fb)], 1)
    ones = np.ones((H, 1), np.float32)
    w2t = np.concatenate([ones, (w_ff * w_ff).T, ones, (w_fb * w_fb).T,
                          ones, (w_att * w_att).T, ones, (w_matt * w_matt).T,
                          (w_mp * w_mp).T], 1)
    bc = lambda r: np.ascontiguousarray(
        np.broadcast_to(r[None, :], (L, L)), dtype=np.float32)
    asf = lambda a: np.ascontiguousarray(a, dtype=np.float32)
    return dict(
        c1=c1, c2=c2, rhs1=asf(rhs1), rhs2=asf(rhs2), w2t=asf(w2t),
        mneg1b=bc((m1 - 1) * 1e30), mneg2b=bc((m2 - 1) * 1e30),
        mone1b=bc(1 - m1), mone2b=bc(1 - m2),
        mnegc1=asf(((m1 - 1) * 1e30)[:, None]),
        mnegc2=asf(((m2 - 1) * 1e30)[:, None]),
        invl1=np.full((L, 1), 1.0 / max(len1, EPS), np.float32),
        invl2=np.full((L, 1), 1.0 / max(len2, EPS), np.float32),
    )


def kernel(context_1, mask_1, context_2, mask_2,
           w_ff, w_fb, w_mp, w_att, w_matt, **_unused):
    context_1 = np.asarray(context_1, dtype=np.float32)
    context_2 = np.asarray(context_2, dtype=np.float32)
    mask_1 = np.asarray(mask_1, dtype=np.float32)
    mask_2 = np.asarray(mask_2, dtype=np.float32)
    w_ff, w_fb = np.asarray(w_ff, np.float32), np.asarray(w_fb, np.float32)
    w_mp = np.asarray(w_mp, np.float32)
    w_att, w_matt = np.asarray(w_att, np.float32), np.asarray(w_matt, np.float32)
    assert context_1.shape == (B, L, H), context_1.shape

    nc = _build()
    in_maps = [
        _host_prep(context_1[b], mask_1[b], context_2[b], mask_2[b],
                   w_ff, w_fb, w_mp, w_att, w_matt)
        for b in range(B)
    ]
    res = run_bass_kernel_spmd(nc, in_maps, core_ids=list(range(B)))
    global LAST_RESULTS
    LAST_RESULTS = res
    return np.stack([res.results[b]["out"] for b in range(B)]).astype(np.float32)


LAST_RESULTS = None


if __name__ == "__main__":
    # smoke test with random data
    rng = np.random.default_rng(0)
    ins = dict(
        context_1=rng.standard_normal((B, L, H), dtype=np.float32),
        context_2=rng.standard_normal((B, L, H), dtype=np.float32),
        mask_1=(np.arange(L)[None, :] < rng.integers(64, 129, B)[:, None]
                ).astype(np.float32),
        mask_2=(np.arange(L)[None, :] < rng.integers(64, 129, B)[:, None]
                ).astype(np.float32),
        w_ff=rng.standard_normal((PP, H), dtype=np.float32) * 0.05,
        w_fb=rng.standard_normal((PP, H), dtype=np.float32) * 0.05,
        w_mp=rng.standard_normal((PP, H), dtype=np.float32) * 0.05,
        w_att=rng.standard_normal((PP, H), dtype=np.float32) * 0.05,
        w_matt=rng.standard_normal((PP, H), dtype=np.float32) * 0.05,
    )
    out = kernel(**ins)
    print("out", out.shape, out.dtype, np.abs(out).max())


# revision 9
# speedup vs baseline: 1.1424x; 1.1424x over previous
"""BiMPM (bilateral multi-perspective matching) kernel for Trainium2.

Contract: kernel(**inputs) takes the FULL unsharded inputs (as produced by
setup_inputs) and returns the full [B, 2L, 102] output. Internally shards
data-parallel over batch B=8 across 8 NeuronCores; the tiny perspective
weights are folded host-side into per-core aux tensors.

Self-contained: hardcodes B=8, L=128, H=768, P=16.
"""
import sys

sys.path.insert(0, "/opt/trn_rl_repo")

import numpy as np
import ml_dtypes
from contextlib import ExitStack

from concourse import bacc, mybir, masks
import concourse.tile as tile
from concourse.bass_utils import run_bass_kernel_spmd
from concourse.bass import MemorySpace

B, L, H, PP, NCH, NF = 8, 128, 768, 16, 6, 102
EPS = 1e-8
F32 = mybir.dt.float32
BF16 = mybir.dt.bfloat16
AX = mybir.AxisListType
OP = mybir.AluOpType
AF = mybir.ActivationFunctionType

# w2t column blocks: [ones|ff16 | ones|fb16 | ones|att16 | ones|matt16 | mp16]
BLK_FF = slice(0, 17)
BLK_FB = slice(17, 34)
BLK_ATT = slice(34, 51)
BLK_MATT = slice(51, 68)
MP0 = 68

# fraction of att-loop iterations whose multiply runs on the Scalar engine
ACT_MOD, ACT_CNT = 16, 9


def _trace_kernel(tc, dins, dout):
    nc = tc.nc
    with ExitStack() as ctx:
        sb = ctx.enter_context(tc.tile_pool(name="sb", bufs=1))
        sc = ctx.enter_context(tc.tile_pool(name="sc", bufs=2))
        tbp = ctx.enter_context(tc.tile_pool(name="tbp", bufs=3))
        ps_t = ctx.enter_context(
            tc.tile_pool(name="ps_t", bufs=4, space=MemorySpace.PSUM))
        ps_w = ctx.enter_context(
            tc.tile_pool(name="ps_w", bufs=2, space=MemorySpace.PSUM))

        # ---- load inputs ----
        def load(name, shape, dt=F32, rearr=None, **kw):
            t = sb.tile(shape, dt, tag=name)
            src = dins[name][:]
            if rearr is not None:
                src = src.rearrange(rearr, **kw)
            nc.sync.dma_start(t[:], src)
            return t

        c1 = load("c1", [L, H])
        c2 = load("c2", [L, H])
        w2t = load("w2t", [L, NCH, 84], rearr="(c p) n -> p c n", p=L)
        rhs1 = load("rhs1", [L, NCH, 34], BF16, "(c p) n -> p c n", p=L)
        rhs2 = load("rhs2", [L, NCH, 34], BF16, "(c p) n -> p c n", p=L)
        mneg1b = load("mneg1b", [L, L])
        mneg2b = load("mneg2b", [L, L])
        mneg1bb = load("mneg1bb", [L, L], BF16)
        mneg2bb = load("mneg2bb", [L, L], BF16)
        mone1b = load("mone1b", [L, L])
        mone2b = load("mone2b", [L, L])
        mnegc1 = load("mnegc1", [L, 1])
        mnegc2 = load("mnegc2", [L, 1])
        invl1 = load("invl1", [L, 1])
        invl2 = load("invl2", [L, 1])

        ident = sb.tile([L, L], F32, tag="ident")
        masks.make_identity(nc, ident[:])
        identb = sb.tile([L, L], BF16, tag="identb")
        masks.make_identity(nc, identb[:])
        ones_colb = sb.tile([L, 1], BF16, tag="ones_colb")
        nc.vector.memset(ones_colb[:], 1.0)
        ones_col = sb.tile([L, 1], F32, tag="ones_col")
        nc.vector.memset(ones_col[:], 1.0)

        out1 = sb.tile([L, NF], F32, tag="out1")
        out2 = sb.tile([L, NF], F32, tag="out2")

        # bf16 copy of the w2 table (for bf16 matmul rhs)
        w2tb = sb.tile([L, NCH, 84], BF16, tag="w2tb")
        nc.scalar.copy(w2tb[:], w2t[:])

        # ---- c1x/c2x: -1e30 rows at invalid positions (bf16, att loops) ----
        c1xb = sb.tile([L, H], BF16, tag="c1xb")
        nc.vector.tensor_scalar(c1xb[:], c1[:], mnegc1[:, 0:1], None, OP.add)
        c2xb = sb.tile([L, H], BF16, tag="c2xb")
        nc.vector.tensor_scalar(c2xb[:], c2[:], mnegc2[:, 0:1], None, OP.add)

        # ---- transposes + squares ----
        c1T = sb.tile([L, NCH, L], F32, tag="c1T")
        c1Tb = sb.tile([L, NCH, L], BF16, tag="c1Tb")
        c1sqT = sb.tile([L, NCH, L], F32, tag="c1sqT")
        c2T = sb.tile([L, NCH, L], F32, tag="c2T")
        c2Tb = sb.tile([L, NCH, L], BF16, tag="c2Tb")
        c2sqT = sb.tile([L, NCH, L], F32, tag="c2sqT")
        for (src, dT, dTb, dsqT) in ((c1, c1T, c1Tb, c1sqT),
                                     (c2, c2T, c2Tb, c2sqT)):
            for c in range(NCH):
                tp = ps_t.tile([L, L], F32, tag="t")
                nc.tensor.transpose(tp[:], src[:, c * L:(c + 1) * L], ident[:])
                nc.scalar.copy(dT[:, c, :], tp[:])
                nc.vector.tensor_copy(dTb[:, c, :], tp[:])
                nc.scalar.square(dsqT[:, c, :], tp[:])

        # ---- weighted norms -> rw1/rw2 [L,84] (fp32 for accuracy) ----
        def rw_of(sqT, tag):
            wnp = ps_w.tile([L, 84], F32, tag="w")
            for c in range(NCH):
                nc.tensor.matmul(wnp[:], sqT[:, c, :], w2t[:, c, :],
                                 start=(c == 0), stop=(c == NCH - 1))
            rw = sb.tile([L, 84], F32, tag=tag)
            nc.scalar.sqrt(rw[:], wnp[:])
            nc.vector.tensor_scalar(rw[:], rw[:], EPS, None, OP.max)
            nc.vector.reciprocal(rw[:], rw[:])
            return rw

        rw1 = rw_of(c1sqT, "rw1")
        rw2 = rw_of(c2sqT, "rw2")

        # ---- ff/bf matvec features -> out[:, 2:36] ----
        def ff_feats(cTb, rhs, rw, out):
            ffp = ps_w.tile([L, 34], F32, tag="w")
            for c in range(NCH):
                nc.tensor.matmul(ffp[:], cTb[:, c, :], rhs[:, c, :],
                                 start=(c == 0), stop=(c == NCH - 1))
            nc.vector.tensor_tensor(out[:, 2:36], ffp[:], rw[:, 0:34], op=OP.mult)

        ff_feats(c1Tb, rhs1, rw1, out1)
        ff_feats(c2Tb, rhs2, rw2, out2)

        # ---- cos chain (fp32) ----
        dotsp = ps_t.tile([L, L], F32, tag="t")
        for c in range(NCH):
            nc.tensor.matmul(dotsp[:], c1T[:, c, :], c2T[:, c, :],
                             start=(c == 0), stop=(c == NCH - 1))
        wS = sc.tile([L, L], F32, tag="wS")
        nc.vector.tensor_scalar(wS[:], dotsp[:], rw1[:, 0:1], None, OP.mult)
        wTp = ps_t.tile([L, L], F32, tag="t")
        nc.tensor.transpose(wTp[:], wS[:], ident[:])
        cosT = sb.tile([L, L], F32, tag="cosT")
        nc.vector.tensor_scalar(cosT[:], wTp[:], rw2[:, 0:1], None, OP.mult)
        cosp = ps_t.tile([L, L], F32, tag="t")
        nc.tensor.transpose(cosp[:], cosT[:], ident[:])
        cos = sb.tile([L, L], F32, tag="cos")
        nc.scalar.copy(cos[:], cosp[:])

        # ---- cmax / cmean -> out[:, 0:2] ----
        def cmaxmean(cosA, cosB, mnegb, invl, out):
            t = sc.tile([L, L], F32, tag="cm")
            nc.vector.tensor_tensor(t[:], cosA[:], mnegb[:], op=OP.add)
            nc.vector.reduce_max(out[:, 0:1], t[:], axis=AX.X)
            mp = ps_t.tile([L, 1], F32, tag="t")
            nc.tensor.matmul(mp[:], cosB[:], ones_col[:], start=True, stop=True)
            nc.vector.tensor_scalar(out[:, 1:2], mp[:], invl[:, 0:1], None, OP.mult)

        cmaxmean(cos, cosT, mneg2b, invl2, out1)
        cmaxmean(cosT, cos, mneg1b, invl1, out2)

        # ---- cosM / cosMT (1.0 in invalid columns, for att-max loops) ----
        cosM = sb.tile([L, L], F32, tag="cosM")
        nc.vector.tensor_tensor(cosM[:], cos[:], mone2b[:], op=OP.add)
        cosMT = sb.tile([L, L], F32, tag="cosMT")
        nc.vector.tensor_tensor(cosMT[:], cosT[:], mone1b[:], op=OP.add)

        # ---- attentive mean (softmax over H of cos @ ctx) -> bf16 ----
        def att_mean(lhsT, rhs, tag):
            sp = ps_w.tile([L, H], F32, tag="w")
            nc.tensor.matmul(sp[:, 0:512], lhsT[:], rhs[:, 0:512],
                             start=True, stop=True)
            nc.tensor.matmul(sp[:, 512:H], lhsT[:], rhs[:, 512:H],
                             start=True, stop=True)
            mx = sc.tile([L, 1], F32, tag="mx")
            nc.vector.reduce_max(mx[:], sp[:], axis=AX.X)
            ngm = sc.tile([L, 1], F32, tag="ngm")
            nc.scalar.mul(ngm[:], mx[:], -1.0)
            am = sb.tile([L, H], BF16, tag=tag)
            se = sc.tile([L, 1], F32, tag="se")
            nc.scalar.activation(am[:], sp[:], AF.Exp, bias=ngm[:, 0:1],
                                 scale=1.0, accum_out=se[:, 0:1])
            rse = sc.tile([L, 1], F32, tag="rse")
            nc.vector.reciprocal(rse[:], se[:])
            nc.vector.tensor_scalar(am[:], am[:], rse[:, 0:1], None, OP.mult)
            return am

        am2 = att_mean(cosT, c2, "am2")   # [i,H]
        am1 = att_mean(cos, c1, "am1")    # [j,H]

        # ---- attentive max loops (PE row-broadcast; mul on ACT or DVE) ----
        def att_max(cxb, cosMcols, tag):
            acc = sb.tile([L, H], BF16, tag=tag)
            nc.vector.memset(acc[:], -1e30)
            for j in range(L):
                bc = ps_w.tile([L, H], F32, tag="w")
                sel = identb[:, j:j + 1].to_broadcast([L, L])
                nc.tensor.matmul(bc[:, 0:512], sel, cxb[:, 0:512],
                                 start=True, stop=True)
                nc.tensor.matmul(bc[:, 512:H], sel, cxb[:, 512:H],
                                 start=True, stop=True)
                if j % ACT_MOD < ACT_CNT:
                    tb = tbp.tile([L, H], BF16, tag="t")
                    nc.scalar.mul(tb[:], bc[:], cosMcols[:, j:j + 1])
                    nc.vector.tensor_tensor(acc[:], acc[:], tb[:], op=OP.max)
                else:
                    nc.vector.scalar_tensor_tensor(
                        acc[:], bc[:], cosMcols[:, j:j + 1], acc[:],
                        OP.mult, OP.max)
            return acc

        amx2 = att_max(c2xb, cosM, "amx2")    # [i,H]
        amx1 = att_max(c1xb, cosMT, "amx1")   # [j,H]

        # ---- mm (pairwise multi-perspective) block, bf16 ----
        for p in range(PP):
            wcol = w2t[:, :, MP0 + p:MP0 + p + 1]
            wc1Tb = sc.tile([L, NCH, L], BF16, tag="wc1Tb")
            for c in range(NCH):
                nc.vector.tensor_scalar(wc1Tb[:, c, :], c1Tb[:, c, :],
                                        w2t[:, c, MP0 + p:MP0 + p + 1],
                                        None, OP.mult)
            nump = ps_t.tile([L, L], F32, tag="t")
            for c in range(NCH):
                nc.tensor.matmul(nump[:], wc1Tb[:, c, :], c2Tb[:, c, :],
                                 start=(c == 0), stop=(c == NCH - 1))
            numSb = sc.tile([L, L], BF16, tag="numSb")
            nc.scalar.copy(numSb[:], nump[:])
            numTp = ps_t.tile([L, L], BF16, tag="t")
            nc.tensor.transpose(numTp[:], numSb[:], identb[:])
            # side 1: scale cols by rnpc2 (via transposed), max/mean over j
            uTb = sc.tile([L, L], BF16, tag="uTb")
            nc.vector.tensor_scalar(uTb[:], numTp[:], rw2[:, MP0 + p:MP0 + p + 1],
                                    None, OP.mult)
            up = ps_t.tile([L, L], BF16, tag="t")
            nc.tensor.transpose(up[:], uTb[:], identb[:])
            tm1 = sc.tile([L, L], BF16, tag="tm1")
            nc.vector.tensor_tensor(tm1[:], up[:], mneg2bb[:], op=OP.add)
            m1r = sc.tile([L, 1], F32, tag="m1r")
            nc.vector.reduce_max(m1r[:], tm1[:], axis=AX.X)
            nc.vector.tensor_scalar(out1[:, 36 + p:37 + p], m1r[:],
                                    rw1[:, MP0 + p:MP0 + p + 1], None, OP.mult)
            mn1 = ps_t.tile([L, 1], F32, tag="t")
            nc.tensor.matmul(mn1[:], uTb[:], ones_colb[:], start=True, stop=True)
            nc.vector.tensor_scalar(out1[:, 52 + p:53 + p], mn1[:],
                                    rw1[:, MP0 + p:MP0 + p + 1], invl2[:, 0:1],
                                    OP.mult, OP.mult)
            # side 2: scale rows by rnpc1, transpose, max/mean over i
            vSb = sc.tile([L, L], BF16, tag="vSb")
            nc.vector.tensor_scalar(vSb[:], numSb[:], rw1[:, MP0 + p:MP0 + p + 1],
                                    None, OP.mult)
            vTp = ps_t.tile([L, L], BF16, tag="t")
            nc.tensor.transpose(vTp[:], vSb[:], identb[:])
            tm2 = sc.tile([L, L], BF16, tag="tm2")
            nc.vector.tensor_tensor(tm2[:], vTp[:], mneg1bb[:], op=OP.add)
            m2r = sc.tile([L, 1], F32, tag="m2r")
            nc.vector.reduce_max(m2r[:], tm2[:], axis=AX.X)
            nc.vector.tensor_scalar(out2[:, 36 + p:37 + p], m2r[:],
                                    rw2[:, MP0 + p:MP0 + p + 1], None, OP.mult)
            mn2 = ps_t.tile([L, 1], F32, tag="t")
            nc.tensor.matmul(mn2[:], vSb[:], ones_colb[:], start=True, stop=True)
            nc.vector.tensor_scalar(out2[:, 52 + p:53 + p], mn2[:],
                                    rw2[:, MP0 + p:MP0 + p + 1], invl1[:, 0:1],
                                    OP.mult, OP.mult)

        # ---- am/amx rowwise mpm feature blocks ----
        def mpm_block(v, cTb, rw_side, blk, out, col0):
            vTb = sc.tile([L, NCH, L], BF16, tag="vTb")
            vsqTb = sc.tile([L, NCH, L], BF16, tag="vsqTb")
            prTb = sc.tile([L, NCH, L], BF16, tag="prTb")
            for c in range(NCH):
                tp = ps_t.tile([L, L], BF16, tag="t")
                nc.tensor.transpose(tp[:], v[:, c * L:(c + 1) * L], identb[:])
                nc.scalar.copy(vTb[:, c, :], tp[:])
                nc.scalar.square(vsqTb[:, c, :], tp[:])
                nc.vector.tensor_tensor(prTb[:, c, :], cTb[:, c, :],
                                        vTb[:, c, :], op=OP.mult)
            nump = ps_w.tile([L, 17], F32, tag="w")
            wnp = ps_w.tile([L, 17], F32, tag="w")
            for c in range(NCH):
                nc.tensor.matmul(nump[:], prTb[:, c, :], w2tb[:, c, blk],
                                 start=(c == 0), stop=(c == NCH - 1))
            for c in range(NCH):
                nc.tensor.matmul(wnp[:], vsqTb[:, c, :], w2tb[:, c, blk],
                                 start=(c == 0), stop=(c == NCH - 1))
            rwv = sc.tile([L, 17], F32, tag="rwv")
            nc.scalar.sqrt(rwv[:], wnp[:])
            nc.vector.tensor_scalar(rwv[:], rwv[:], EPS, None, OP.max)
            nc.vector.reciprocal(rwv[:], rwv[:])
            ft = sc.tile([L, 17], F32, tag="ft")
            nc.vector.tensor_tensor(ft[:], nump[:], rw_side[:, blk], op=OP.mult)
            nc.vector.tensor_tensor(out[:, col0:col0 + 17], ft[:], rwv[:],
                                    op=OP.mult)

        mpm_block(am2, c1Tb, rw1, BLK_ATT, out1, 68)
        mpm_block(am1, c2Tb, rw2, BLK_ATT, out2, 68)
        mpm_block(amx2, c1Tb, rw1, BLK_MATT, out1, 85)
        mpm_block(amx1, c2Tb, rw2, BLK_MATT, out2, 85)

        # ---- store ----
        nc.sync.dma_start(dout[0:L, :], out1[:])
        nc.sync.dma_start(dout[L:2 * L, :], out2[:])


_CACHED = None


def _build():
    global _CACHED
    if _CACHED is not None:
        return _CACHED
    nc = bacc.Bacc("TRN2", target_bir_lowering=False, debug=False,
                   enable_asserts=False)
    dins = {}
    for name, shape, dt in [
            ("c1", [L, H], F32), ("c2", [L, H], F32),
            ("rhs1", [H, 34], BF16), ("rhs2", [H, 34], BF16),
            ("w2t", [H, 84], F32),
            ("mneg1b", [L, L], F32), ("mneg2b", [L, L], F32),
            ("mneg1bb", [L, L], BF16), ("mneg2bb", [L, L], BF16),
            ("mone1b", [L, L], F32), ("mone2b", [L, L], F32),
            ("mnegc1", [L, 1], F32), ("mnegc2", [L, 1], F32),
            ("invl1", [L, 1], F32), ("invl2", [L, 1], F32)]:
        dins[name] = nc.dram_tensor(name, shape, dt, kind="ExternalInput")
    dout = nc.dram_tensor("out", [2 * L, NF], F32, kind="ExternalOutput")
    with tile.TileContext(nc) as tc:
        _trace_kernel(tc, dins, dout[:])
    nc.compile()
    _CACHED = nc
    return nc


def _host_prep(c1raw, m1, c2raw, m2, w_ff, w_fb, w_mp, w_att, w_matt):
    c1 = (c1raw * m1[:, None]).astype(np.float32)
    c2 = (c2raw * m2[:, None]).astype(np.float32)
    len1, len2 = float(m1.sum()), float(m2.sum())
    lp1, lp2 = max(int(len1) - 1, 0), max(int(len2) - 1, 0)

    def mpm_rhs(v, w):
        w2 = w * w
        rn = 1.0 / max(np.sqrt((v * v).sum()), EPS)
        wn = np.sqrt((w2 * (v * v)[None, :]).sum(1))
        rwn = 1.0 / np.maximum(wn, EPS)
        return np.concatenate(
            [(v * rn)[:, None], (w2 * v[None, :] * rwn[:, None]).T], 1)

    rhs1 = np.concatenate([mpm_rhs(c2[lp2], w_ff), mpm_rhs(c2[0], w_fb)], 1)
    rhs2 = np.concatenate([mpm_rhs(c1[lp1], w_ff), mpm_rhs(c1[0], w_fb)], 1)
    ones = np.ones((H, 1), np.float32)
    w2t = np.concatenate([ones, (w_ff * w_ff).T, ones, (w_fb * w_fb).T,
                          ones, (w_att * w_att).T, ones, (w_matt * w_matt).T,
                          (w_mp * w_mp).T], 1)
    bc = lambda r: np.ascontiguousarray(
        np.broadcast_to(r[None, :], (L, L)), dtype=np.float32)
    asf = lambda a: np.ascontiguousarray(a, dtype=np.float32)
    asb = lambda a: np.ascontiguousarray(a, dtype=ml_dtypes.bfloat16)
    mneg1 = bc((m1 - 1) * 1e30)
    mneg2 = bc((m2 - 1) * 1e30)
    return dict(
        c1=c1, c2=c2, rhs1=asb(rhs1), rhs2=asb(rhs2), w2t=asf(w2t),
        mneg1b=mneg1, mneg2b=mneg2,
        mneg1bb=asb(mneg1), mneg2bb=asb(mneg2),
        mone1b=bc(1 - m1), mone2b=bc(1 - m2),
        mnegc1=asf(((m1 - 1) * 1e30)[:, None]),
        mnegc2=asf(((m2 - 1) * 1e30)[:, None]),
        invl1=np.full((L, 1), 1.0 / max(len1, EPS), np.float32),
        invl2=np.full((L, 1), 1.0 / max(len2, EPS), np.float32),
    )


def kernel(context_1, mask_1, context_2, mask_2,
           w_ff, w_fb, w_mp, w_att, w_matt, **_unused):
    context_1 = np.asarray(context_1, dtype=np.float32)
    context_2 = np.asarray(context_2, dtype=np.float32)
    mask_1 = np.asarray(mask_1, dtype=np.float32)
    mask_2 = np.asarray(mask_2, dtype=np.float32)
    w_ff, w_fb = np.asarray(w_ff, np.float32), np.asarray(w_fb, np.float32)
    w_mp = np.asarray(w_mp, np.float32)
    w_att, w_matt = np.asarray(w_att, np.float32), np.asarray(w_matt, np.float32)
    assert context_1.shape == (B, L, H), context_1.shape

    nc = _build()
    in_maps = [
        _host_prep(context_1[b], mask_1[b], context_2[b], mask_2[b],
                   w_ff, w_fb, w_mp, w_att, w_matt)
        for b in range(B)
    ]
    res = run_bass_kernel_spmd(nc, in_maps, core_ids=list(range(B)))
    global LAST_RESULTS
    LAST_RESULTS = res
    return np.stack([res.results[b]["out"] for b in range(B)]).astype(np.float32)


LAST_RESULTS = None


if __name__ == "__main__":
    rng = np.random.default_rng(0)
    ins = dict(
        context_1=rng.standard_normal((B, L, H), dtype=np.float32),
        context_2=rng.standard_normal((B, L, H), dtype=np.float32),
        mask_1=(np.arange(L)[None, :] < rng.integers(64, 129, B)[:, None]
                ).astype(np.float32),
        mask_2=(np.arange(L)[None, :] < rng.integers(64, 129, B)[:, None]
                ).astype(np.float32),
        w_ff=rng.standard_normal((PP, H), dtype=np.float32) * 0.05,
        w_fb=rng.standard_normal((PP, H), dtype=np.float32) * 0.05,
        w_mp=rng.standard_normal((PP, H), dtype=np.float32) * 0.05,
        w_att=rng.standard_normal((PP, H), dtype=np.float32) * 0.05,
        w_matt=rng.standard_normal((PP, H), dtype=np.float32) * 0.05,
    )
    out = kernel(**ins)
    print("out", out.shape, out.dtype, np.abs(out).max())
